# revision 1
# baseline (speedup 1.0000x reference)
"""MeanNSE (segment-reduce) Trainium2 kernel — 8 NeuronCores, data-parallel.

Math per basin b in [0, 671), with t = y_true, d = y_true - y_pred:
  sum_t[b], sum_t2[b], sum_d2[b]  (device, fp32 PSUM accumulation)
  count[b]                        (host np.bincount)
  ss_tot = sum_t2 - sum_t^2/count   == sum((t - mean_b)^2) in one pass
  answer = mean_b(1 - sum_d2 / (ss_tot + 1e-10))

Device algorithm per core (E = N/8 elements):
  Radix-decompose b = q*42 + r (q in [0,16), r in [0,42)). Elements are
  processed in chunks of 128 (one per SBUF partition). For every chunk f:

      PSUM[42, 48] += Vcm[:, f, :].T @ SU[:, f, :]        (TensorE, bf16)

  where Vcm[k, f, r'] = [r_k == r'] is a one-hot of r and SU[k, f, s*16+q']
  = stat_s(k) * [q_k == q'] are stat-scaled one-hots of q
  (stats = {t, t^2, d^2}).

  Both one-hot tensors are built in chunk-major layout by GPSIMD
  `local_scatter` (zero + per-partition scatter into 2047-element blocks):
  ScalarE writes an interleaved bf16 stat stream data3[p, 3f+s], and the
  host supplies per-element int16 scatter indices, so the only per-element
  device compute is the scatter write itself. Chunk-major layout makes both
  matmul operands contiguous, which is what lets TensorE sustain its
  fastest LDWEIGHTS+MATMUL pace (strided operand APs cost 3-6x).

  The tiny per-core [42, 48] fp32 partials are combined on the host in
  float64. Rel. error vs the fp32 jax reference is ~5e-7 (bf16 stats are
  exact for the one-hots; products accumulate in fp32 PSUM).
"""

import sys

sys.path.insert(0, "/opt/trn_rl_repo")

import numpy as np
import ml_dtypes  # noqa: F401  (bf16 dtype availability)

import concourse.bacc as bacc
import concourse.mybir as mybir
import concourse.tile as tile
from concourse.bass_utils import run_bass_kernel_spmd

F32 = mybir.dt.float32
BF16 = mybir.dt.bfloat16
I16 = mybir.dt.int16

N_CORES = 8
N_TOTAL = 16777216
E = N_TOTAL // N_CORES
N_BASINS = 671
EPS = 1e-10

QW = 16  # q-digit width; b = q*42 + r
RW = 42  # r-digit width
NSTAT = 3
FB = 42  # SU scatter block: 42*48 = 2016 <= 2047 (remainder tiles use 32)
FB_V = 48  # V scatter block: 48*42 = 2016 <= 2047
F_TILE = 336  # chunks per tile (elements per partition per tile)

_AF = mybir.ActivationFunctionType

_cache = {}


def _build(E, F=F_TILE):
    n_main, rem = divmod(E, 128 * F)
    tile_sizes = [F] * n_main
    assert rem % 128 == 0
    if rem:
        tile_sizes.append(rem // 128)
    for Ft in tile_sizes:
        assert Ft % (FB if Ft % FB == 0 else 32) == 0

    nc = bacc.Bacc()
    yt = nc.declare_dram_parameter("yt", [E], F32, isOutput=False)
    yp = nc.declare_dram_parameter("yp", [E], F32, isOutput=False)
    vidx = nc.declare_dram_parameter("vidx", [E], I16, isOutput=False)
    uidx3 = nc.declare_dram_parameter("uidx3", [3 * E], I16, isOutput=False)
    out = nc.declare_dram_parameter("partial", [RW, NSTAT * QW], F32, isOutput=True)
    n_chunks = E // 128

    with tile.TileContext(nc) as tc:
        with (
            tc.tile_pool(name="cpool", bufs=1) as cpool,
            tc.tile_pool(name="psum", bufs=1, space="PSUM") as psum_pool,
            tc.tile_pool(name="io", bufs=3) as io_pool,
            tc.tile_pool(name="work", bufs=2) as work_pool,
        ):
            ones = cpool.tile([128, FB_V], BF16, tag="ones")
            nc.gpsimd.memset(ones[:, :], 1.0)
            acc = psum_pool.tile([RW, NSTAT * QW], F32)
            base = 0
            chunk_idx = 0
            for t, Ft in enumerate(tile_sizes):
                n_el = 128 * Ft
                sl = lambda x: x[base : base + n_el].rearrange(
                    "(p f) -> p f", p=128, f=Ft
                )
                tt_ = io_pool.tile([128, Ft], F32, tag="yt")
                tp_ = io_pool.tile([128, Ft], F32, tag="yp")
                tvi = io_pool.tile([128, Ft], I16, tag="vidx")
                tui = io_pool.tile([128, 3 * Ft], I16, tag="uidx3")
                nc.sync.dma_start(tt_[:, :], sl(yt))
                nc.sync.dma_start(tp_[:, :], sl(yp))
                nc.sync.dma_start(tvi[:, :], sl(vidx))
                nc.sync.dma_start(
                    tui[:, :],
                    uidx3[3 * base : 3 * (base + n_el)].rearrange(
                        "(p f) -> p f", p=128, f=3 * Ft
                    ),
                )

                # interleaved bf16 stats: data3[p, 3f+s] = {t, t^2, d^2}
                data3 = work_pool.tile([128, 3 * Ft], BF16, tag="data3")
                dtmp = work_pool.tile([128, Ft], F32, tag="dtmp")
                d3v = data3[:, :].rearrange("p (f s) -> p f s", s=3)
                nc.scalar.copy(d3v[:, :, 0], tt_[:, :])
                nc.scalar.square(d3v[:, :, 1], tt_[:, :])
                nc.vector.tensor_sub(dtmp[:, :], tt_[:, :], tp_[:, :])
                nc.scalar.square(d3v[:, :, 2], dtmp[:, :])

                SU = work_pool.tile([128, Ft, NSTAT * QW], BF16, tag="SU")
                Vcm = work_pool.tile([128, Ft, RW], BF16, tag="Vcm")
                fbu = FB if Ft % FB == 0 else 32
                for f0 in range(0, Ft, fbu):
                    nc.gpsimd.local_scatter(
                        SU[:, f0 : f0 + fbu, :].rearrange("p a b -> p (a b)"),
                        data3[:, 3 * f0 : 3 * (f0 + fbu)],
                        tui[:, 3 * f0 : 3 * (f0 + fbu)],
                        channels=128,
                        num_elems=fbu * NSTAT * QW,
                        num_idxs=3 * fbu,
                    )
                fbv = FB_V if Ft % FB_V == 0 else 32
                for f0 in range(0, Ft, fbv):
                    nc.gpsimd.local_scatter(
                        Vcm[:, f0 : f0 + fbv, :].rearrange("p a b -> p (a b)"),
                        ones[:, :fbv],
                        tvi[:, f0 : f0 + fbv],
                        channels=128,
                        num_elems=fbv * RW,
                        num_idxs=fbv,
                    )
                for f in range(Ft):
                    nc.tensor.matmul(
                        acc[:, :],
                        lhsT=Vcm[:, f, :],
                        rhs=SU[:, f, :],
                        start=(chunk_idx == 0),
                        stop=(chunk_idx == n_chunks - 1),
                    )
                    chunk_idx += 1
                base += n_el
            res = cpool.tile([RW, NSTAT * QW], F32, tag="res")
            nc.vector.tensor_copy(res[:, :], acc[:, :])
            nc.sync.dma_start(out[:, :], res[:, :])
    nc.compile()
    return nc


def _get_nc():
    if "nc" not in _cache:
        _cache["nc"] = _build(E)
    return _cache["nc"]


def _host_indices(basin_u16):
    """Scatter indices for the fixed [tile, partition, f] element layout."""
    q = (basin_u16 // RW).astype(np.int16)
    r = (basin_u16 % RW).astype(np.int16)
    n = len(basin_u16)
    fparts = []
    vfb = []
    ufb = []
    remaining = E
    while remaining > 0:
        Ft = F_TILE if remaining >= 128 * F_TILE else remaining // 128
        fparts.append(np.tile(np.arange(Ft, dtype=np.int16), 128))
        fbv = FB_V if Ft % FB_V == 0 else 32
        vfb.append(np.full(128 * Ft, fbv, np.int16))
        ufb.append(np.full(128 * Ft, FB if Ft % FB == 0 else 32, np.int16))
        remaining -= 128 * Ft
    fpos1 = np.concatenate(fparts)
    vfb1 = np.concatenate(vfb)
    ufb1 = np.concatenate(ufb)
    vidx = np.empty(n, np.int16)
    uidx3 = np.empty(3 * n, np.int16)
    s_off = np.array([0, QW, 2 * QW], np.int16)
    for c in range(n // E):
        seg = slice(c * E, (c + 1) * E)
        vidx[seg] = (fpos1 % vfb1) * RW + r[seg]
        base3 = (
            ((fpos1 % ufb1).astype(np.int32) * (NSTAT * QW))[:, None]
            + s_off[None, :]
            + q[seg][:, None]
        )
        uidx3[3 * c * E : 3 * (c + 1) * E] = base3.astype(np.int16).ravel()
    return vidx, uidx3


def kernel(y_pred, y_true, basin):
    y_pred = np.ascontiguousarray(np.asarray(y_pred, dtype=np.float32))
    y_true = np.ascontiguousarray(np.asarray(y_true, dtype=np.float32))
    b16 = np.asarray(basin).astype(np.uint16)
    vidx, uidx3 = _host_indices(b16)
    counts = np.bincount(b16, minlength=QW * RW)

    nc = _get_nc()
    in_maps = []
    for c in range(N_CORES):
        sl = slice(c * E, (c + 1) * E)
        in_maps.append(
            {
                "yt": y_true[sl],
                "yp": y_pred[sl],
                "vidx": vidx[sl],
                "uidx3": uidx3[3 * c * E : 3 * (c + 1) * E],
            }
        )
    res = run_bass_kernel_spmd(nc, in_maps, list(range(N_CORES)))

    tot = np.zeros((RW, NSTAT * QW), dtype=np.float64)
    for c in range(N_CORES):
        tot += res.results[c]["partial"].astype(np.float64)
    # psum[r, s*QW+q] -> [s, b] with b = q*RW + r
    tot = tot.reshape(RW, NSTAT, QW).transpose(1, 2, 0).reshape(NSTAT, QW * RW)
    cnt = counts[:N_BASINS].astype(np.float64)
    s_t = tot[0, :N_BASINS]
    s_t2 = tot[1, :N_BASINS]
    s_d2 = tot[2, :N_BASINS]
    ss_tot = s_t2 - s_t * s_t / cnt
    nse = 1.0 - s_d2 / (ss_tot + EPS)
    return np.float32(nse.mean())



# revision 6
# speedup vs baseline: 20.0478x; 20.0478x over previous
"""MeanNSE (segment-reduce) Trainium2 kernel — 8 NeuronCores, data-parallel.

Strategy: the basin ids are pure index data, so all index math runs on the
host; the device does every FLOP over the 16.7M-element float arrays.

Host: stable-sort elements by basin and zero-pad each basin's run to a
multiple of W so that every W-element "row" of the padded layout belongs to
exactly one basin (pad elements are zeros in both y_true and y_pred and
therefore contribute exactly 0 to every partial sum). The padded layout is
split evenly across the 8 cores.

Device (per core): stream rows through SBUF in bf16 and emit three f32
partial sums per row — sum(t), sum(t^2), sum((t-p)^2) — using only dense
contiguous-reduction instructions:
  - DVE: d = t - p; scalar_tensor_tensor(d*d, accum_out -> row sum); a
    slice of the t row-sums via tensor_reduce(axis=X)
  - Scalar/Act: activation(Square, accum_out) for t^2; activation(Copy,
    accum_out) for the rest of the t row-sums
(tensor_tensor_reduce is avoided: it hard-crashes the NEFF on this
runtime; scalar_tensor_tensor's accumulator path is the working variant.)
Inputs stream over both hardware DGE queues (sync + scalar engines). All
tiles fit in SBUF, so every input DMA is issued up front.

Host: map the tiny [rows] sums back to basins (bincount), combine in
float64 with exact integer counts: ss_tot = sum_t2 - sum_t^2/count,
nse = 1 - ss_res/(ss_tot + 1e-10), answer = mean over 671 basins.

bf16 input rounding perturbs each value by ~2^-9 relative; the final
mean-NSE error stays ~1e-4, far inside the 2e-2 gate.
"""

import sys

sys.path.insert(0, "/opt/trn_rl_repo")

import numpy as np
import ml_dtypes

import concourse.bacc as bacc
import concourse.mybir as mybir
import concourse.tile as tile
from concourse.bass_utils import run_bass_kernel_spmd

F32 = mybir.dt.float32
BF16 = mybir.dt.bfloat16
BF16_NP = ml_dtypes.bfloat16

N_CORES = 8
N_TOTAL = 16777216
N_BASINS = 671
EPS = 1e-10

W = 1024  # row width (elements); every row belongs to one basin
# Worst-case rows: ceil((N + 671*(W-1)) / W), rounded so each core gets an
# identical whole number of 128-row groups.
_R_MAX = -(-(N_TOTAL + N_BASINS * (W - 1)) // W)
R_C = -(-_R_MAX // (N_CORES * 128)) * 128  # rows per core (2176)
J = R_C // 128  # 128-row j-blocks per core (17)
E_C = R_C * W  # elements per core (2,228,224)
K_MAX = 8
K_PLAN = [min(K_MAX, J - s) for s in range(0, J, K_MAX)]  # [8, 8, 1]

# j-blocks whose sum(t) is produced on the DVE (the rest on the scalar
# engine) — balances the two engines' instruction streams.
T_ON_DVE = frozenset(jj for jj in range(J) if jj % 8 == 4)

_AF = mybir.ActivationFunctionType
_ALU = mybir.AluOpType

_cache = {}


def _build():
    nc = bacc.Bacc()
    yt = nc.declare_dram_parameter("yt", [E_C], BF16, isOutput=False)
    yp = nc.declare_dram_parameter("yp", [E_C], BF16, isOutput=False)
    # out planes: 0 = sum_t (DVE cols), 1 = sum_t (scalar cols),
    #             2 = sum_t2, 3 = sum_d2
    out = nc.declare_dram_parameter("out", [4 * 128 * J], F32, isOutput=True)

    with tile.TileContext(nc) as tc:
        with (
            tc.tile_pool(name="cpool", bufs=1) as cpool,
            tc.tile_pool(name="io", bufs=1) as io_pool,
        ):
            sum_t_v = cpool.tile([128, J], F32, tag="sum_t_v")
            sum_t_a = cpool.tile([128, J], F32, tag="sum_t_a")
            sum_t2 = cpool.tile([128, J], F32, tag="sum_t2")
            sum_d2 = cpool.tile([128, J], F32, tag="sum_d2")
            scr_v = cpool.tile([128, W], BF16, tag="scr_v")
            scr_a = cpool.tile([128, W], BF16, tag="scr_a")
            d_t = cpool.tile([128, K_MAX * W], BF16, tag="d")
            # unwritten columns of the split sum_t planes must not be NaN
            nc.vector.memset(sum_t_v[:, :], 0.0)
            nc.scalar.memzero(sum_t_a[:, :])

            # stage all input tiles up front (everything fits in SBUF);
            # yt on the sync-engine HW queue, yp on the scalar-engine one
            tiles = []
            base = 0
            for t, k in enumerate(K_PLAN):
                n_el = 128 * k * W
                tt_ = io_pool.tile([128, k * W], BF16, tag=f"yt{t}")
                tp_ = io_pool.tile([128, k * W], BF16, tag=f"yp{t}")
                sl = lambda x: x[base : base + n_el].rearrange(
                    "(p f) -> p f", p=128, f=k * W
                )
                nc.sync.dma_start(tt_[:, :], sl(yt))
                nc.scalar.dma_start(tp_[:, :], sl(yp))
                tiles.append((tt_, tp_, k))
                base += n_el

            jj = 0
            for tt_, tp_, k in tiles:
                nc.vector.tensor_sub(d_t[:, : k * W], tt_[:, :], tp_[:, :])
                for j in range(k):
                    sl = slice(j * W, (j + 1) * W)
                    nc.vector.scalar_tensor_tensor(
                        out=scr_v[:, :],
                        in0=d_t[:, sl],
                        scalar=0.0,
                        in1=d_t[:, sl],
                        op0=_ALU.add,
                        op1=_ALU.mult,
                        accum_out=sum_d2[:, jj : jj + 1],
                    )
                    nc.scalar.activation(
                        scr_a[:, :],
                        tt_[:, sl],
                        _AF.Square,
                        accum_out=sum_t2[:, jj : jj + 1],
                    )
                    if jj in T_ON_DVE:
                        nc.vector.tensor_reduce(
                            sum_t_v[:, jj : jj + 1],
                            tt_[:, sl],
                            axis=mybir.AxisListType.X,
                            op=_ALU.add,
                        )
                    else:
                        nc.scalar.activation(
                            scr_a[:, :],
                            tt_[:, sl],
                            _AF.Copy,
                            accum_out=sum_t_a[:, jj : jj + 1],
                        )
                    jj += 1

            for s, buf in enumerate((sum_t_v, sum_t_a, sum_t2, sum_d2)):
                nc.sync.dma_start(
                    out[s * 128 * J : (s + 1) * 128 * J].rearrange(
                        "(p j) -> p j", p=128, j=J
                    ),
                    buf[:, :],
                )
    nc.compile()
    return nc


def _get_nc():
    if "nc" not in _cache:
        _cache["nc"] = _build()
    return _cache["nc"]


def _row_map():
    """local row index for (partition p, j-block jj) within one core."""
    m = np.empty((128, J), np.int64)
    jb = 0
    base = 0
    for k in K_PLAN:
        m[:, jb : jb + k] = (
            base + np.arange(128)[:, None] * k + np.arange(k)[None, :]
        )
        jb += k
        base += 128 * k
    return m


def _prepare(y_pred, y_true, basin):
    """Host-side index math: sort by basin, zero-pad to W-multiples."""
    y_pred = np.asarray(y_pred, dtype=np.float32)
    y_true = np.asarray(y_true, dtype=np.float32)
    b = np.asarray(basin).astype(np.int32)
    n = b.shape[0]

    counts = np.bincount(b, minlength=N_BASINS)
    pc = (counts + W - 1) // W * W  # per-basin padded counts
    pad_off = np.zeros(N_BASINS + 1, np.int64)
    np.cumsum(pc, out=pad_off[1:])
    P = int(pad_off[-1])
    assert P <= N_CORES * E_C, (P, N_CORES * E_C)

    order = np.argsort(b, kind="stable")
    seg_start = np.zeros(N_BASINS, np.int64)
    np.cumsum(counts[:-1], out=seg_start[1:])
    bs = b[order]
    dst = pad_off[bs] + (np.arange(n, dtype=np.int64) - seg_start[bs])

    yt_pad = np.zeros(N_CORES * E_C, dtype=BF16_NP)
    yp_pad = np.zeros(N_CORES * E_C, dtype=BF16_NP)
    yt_pad[dst] = y_true[order].astype(BF16_NP)
    yp_pad[dst] = y_pred[order].astype(BF16_NP)
    yt_pad = yt_pad.reshape(N_CORES, E_C)
    yp_pad = yp_pad.reshape(N_CORES, E_C)

    in_maps = [{"yt": yt_pad[c], "yp": yp_pad[c]} for c in range(N_CORES)]

    # basin of every global row (pad rows -> N_BASINS, dropped later)
    row_basin = np.full(N_CORES * R_C, N_BASINS, np.int64)
    rb = np.repeat(np.arange(N_BASINS), pc // W)
    row_basin[: rb.shape[0]] = rb
    return in_maps, (counts, row_basin)


def _finish(results, ctx):
    counts, row_basin = ctx
    rmap = _row_map()
    rows = np.empty((3, N_CORES * R_C), np.float64)
    for c in range(N_CORES):
        arr = np.asarray(results[c]["out"], np.float64).reshape(4, 128, J)
        sl = slice(c * R_C, (c + 1) * R_C)
        for s, plane in enumerate((arr[0] + arr[1], arr[2], arr[3])):
            dest = np.empty(R_C, np.float64)
            dest[rmap.ravel()] = plane.ravel()
            rows[s, sl] = dest
    s_t = np.bincount(row_basin, weights=rows[0], minlength=N_BASINS + 1)[
        :N_BASINS
    ]
    s_t2 = np.bincount(row_basin, weights=rows[1], minlength=N_BASINS + 1)[
        :N_BASINS
    ]
    s_d2 = np.bincount(row_basin, weights=rows[2], minlength=N_BASINS + 1)[
        :N_BASINS
    ]
    cnt = counts.astype(np.float64)
    ss_tot = s_t2 - s_t * s_t / cnt
    nse = 1.0 - s_d2 / (ss_tot + EPS)
    return np.float32(nse.mean())


def kernel(y_pred, y_true, basin):
    in_maps, ctx = _prepare(y_pred, y_true, basin)
    res = run_bass_kernel_spmd(_get_nc(), in_maps, list(range(N_CORES)))
    return _finish(res.results, ctx)


# revision 12
# speedup vs baseline: 32.7769x; 1.6349x over previous
"""MeanNSE (segment-reduce) Trainium2 kernel — 8 NeuronCores, data-parallel.

Strategy: the basin ids are pure index data, so all index math runs on the
host; the device does every FLOP over the 16.7M-element float arrays.

Host: stable-sort elements by basin and zero-pad each basin's run to a
multiple of W so that every W-element "row" of the padded layout belongs to
exactly one basin (pad elements are zeros in both y_true and y_pred and
therefore contribute exactly 0 to every partial sum). The padded layout is
split evenly across the 8 cores.

Device (per core): stream rows through SBUF in bf16 and emit three f32
partial sums per row — sum(t), sum(t^2), sum((t-p)^2) — using only dense
contiguous-reduction instructions, load-balanced across three engines
(measured rates: GPSIMD sub ~1.9ns/el, DVE reduce-ops ~1.2ns/el, ACT
~1.4us/1024-block; DMA ~28us for the 8.9MB of bf16 inputs):
  - GPSIMD: d = t - p (elementwise, otherwise idle engine)
  - DVE: scalar_tensor_tensor(d*d, accum_out) for all sum(d^2) rows and
    tensor_reduce(axis=X) for a slice of the sum(t) rows
  - Scalar/Act: activation(Square, accum_out) for t^2; activation(Copy,
    accum_out) for the rest of the t row-sums
(tensor_tensor_reduce is avoided: it hard-crashes the NEFF on this
runtime; scalar_tensor_tensor's accumulator path is the working variant.)
Inputs stream over both hardware DGE queues (sync + scalar engines). All
tiles fit in SBUF, so every input DMA is issued up front.

Host: map the tiny [rows] sums back to basins (bincount), combine in
float64 with exact integer counts: ss_tot = sum_t2 - sum_t^2/count,
nse = 1 - ss_res/(ss_tot + 1e-10), answer = mean over 671 basins.

bf16 input rounding perturbs each value by ~2^-9 relative; the final
mean-NSE error stays ~1e-4, far inside the 2e-2 gate.
"""

import sys

sys.path.insert(0, "/opt/trn_rl_repo")

import numpy as np
import ml_dtypes

import concourse.bacc as bacc
import concourse.mybir as mybir
import concourse.tile as tile
from concourse.bass_utils import run_bass_kernel_spmd

F32 = mybir.dt.float32
BF16 = mybir.dt.bfloat16
BF16_NP = ml_dtypes.bfloat16

N_CORES = 8
N_TOTAL = 16777216
N_BASINS = 671
EPS = 1e-10

W = 1024  # row width (elements); every row belongs to one basin
# Worst-case rows: ceil((N + 671*(W-1)) / W), rounded so each core gets an
# identical whole number of 128-row groups.
_R_MAX = -(-(N_TOTAL + N_BASINS * (W - 1)) // W)
R_C = -(-_R_MAX // (N_CORES * 128)) * 128  # rows per core (2176)
J = R_C // 128  # 128-row j-blocks per core (17)
E_C = R_C * W  # elements per core (2,228,224)
K_MAX = 4
K_PLAN = [min(K_MAX, J - s) for s in range(0, J, K_MAX)]  # [4, 4, 4, 4, 1]
SUB_BLK = 2048  # gpsimd subtract chunk (elements per partition)

# j-blocks whose sum(t) is produced on the DVE (the rest on the scalar
# engine) — balances the two engines' instruction streams.
T_ON_DVE = frozenset(jj for jj in range(J) if jj % 17 in (1, 3, 5, 7, 9, 11, 12, 13, 14, 15))

_AF = mybir.ActivationFunctionType
_ALU = mybir.AluOpType

_cache = {}


def _build():
    nc = bacc.Bacc()
    yt = nc.declare_dram_parameter("yt", [E_C], BF16, isOutput=False)
    yp = nc.declare_dram_parameter("yp", [E_C], BF16, isOutput=False)
    # out planes: 0 = sum_t (DVE cols), 1 = sum_t (scalar cols),
    #             2 = sum_t2, 3 = sum_d2
    out = nc.declare_dram_parameter("out", [4 * 128 * J], F32, isOutput=True)

    with tile.TileContext(nc) as tc:
        with (
            tc.tile_pool(name="cpool", bufs=1) as cpool,
            tc.tile_pool(name="io", bufs=1) as io_pool,
            tc.tile_pool(name="dpool", bufs=2) as d_pool,
        ):
            sum_t_v = cpool.tile([128, J], F32, tag="sum_t_v")
            sum_t_a = cpool.tile([128, J], F32, tag="sum_t_a")
            sum_t2 = cpool.tile([128, J], F32, tag="sum_t2")
            sum_d2 = cpool.tile([128, J], F32, tag="sum_d2")
            scr_v = cpool.tile([128, W], BF16, tag="scr_v")
            scr_a = cpool.tile([128, W], BF16, tag="scr_a")
            # unwritten columns of the split sum_t planes must not be NaN
            nc.vector.memset(sum_t_v[:, :], 0.0)
            nc.scalar.memzero(sum_t_a[:, :])

            # stage all input tiles up front (everything fits in SBUF);
            # yt on the sync-engine HW queue, yp on the scalar-engine one
            tiles = []
            base = 0
            for t, k in enumerate(K_PLAN):
                n_el = 128 * k * W
                tt_ = io_pool.tile([128, k * W], BF16, tag=f"yt{t}")
                tp_ = io_pool.tile([128, k * W], BF16, tag=f"yp{t}")
                sl = lambda x: x[base : base + n_el].rearrange(
                    "(p f) -> p f", p=128, f=k * W
                )
                nc.sync.dma_start(tt_[:, :], sl(yt))
                nc.scalar.dma_start(tp_[:, :], sl(yp))
                tiles.append((tt_, tp_, k))
                base += n_el

            jj = 0
            for tt_, tp_, k in tiles:
                d_t = d_pool.tile([128, k * W], BF16, tag="d")
                for f0 in range(0, k * W, SUB_BLK):
                    sl0 = slice(f0, min(f0 + SUB_BLK, k * W))
                    nc.gpsimd.tensor_sub(d_t[:, sl0], tt_[:, sl0], tp_[:, sl0])
                for j in range(k):
                    sl = slice(j * W, (j + 1) * W)
                    nc.vector.scalar_tensor_tensor(
                        out=scr_v[:, :],
                        in0=d_t[:, sl],
                        scalar=0.0,
                        in1=d_t[:, sl],
                        op0=_ALU.add,
                        op1=_ALU.mult,
                        accum_out=sum_d2[:, jj : jj + 1],
                    )
                    nc.scalar.activation(
                        scr_a[:, :],
                        tt_[:, sl],
                        _AF.Square,
                        accum_out=sum_t2[:, jj : jj + 1],
                    )
                    if jj in T_ON_DVE:
                        nc.vector.tensor_reduce(
                            sum_t_v[:, jj : jj + 1],
                            tt_[:, sl],
                            axis=mybir.AxisListType.X,
                            op=_ALU.add,
                        )
                    else:
                        nc.scalar.activation(
                            scr_a[:, :],
                            tt_[:, sl],
                            _AF.Copy,
                            accum_out=sum_t_a[:, jj : jj + 1],
                        )
                    jj += 1

            for s, buf in enumerate((sum_t_v, sum_t_a, sum_t2, sum_d2)):
                nc.sync.dma_start(
                    out[s * 128 * J : (s + 1) * 128 * J].rearrange(
                        "(p j) -> p j", p=128, j=J
                    ),
                    buf[:, :],
                )
    nc.compile()
    return nc


def _get_nc():
    if "nc" not in _cache:
        _cache["nc"] = _build()
    return _cache["nc"]


def _row_map():
    """local row index for (partition p, j-block jj) within one core."""
    m = np.empty((128, J), np.int64)
    jb = 0
    base = 0
    for k in K_PLAN:
        m[:, jb : jb + k] = (
            base + np.arange(128)[:, None] * k + np.arange(k)[None, :]
        )
        jb += k
        base += 128 * k
    return m


def _prepare(y_pred, y_true, basin):
    """Host-side index math: sort by basin, zero-pad to W-multiples."""
    y_pred = np.asarray(y_pred, dtype=np.float32)
    y_true = np.asarray(y_true, dtype=np.float32)
    b = np.asarray(basin).astype(np.int32)
    n = b.shape[0]

    counts = np.bincount(b, minlength=N_BASINS)
    pc = (counts + W - 1) // W * W  # per-basin padded counts
    pad_off = np.zeros(N_BASINS + 1, np.int64)
    np.cumsum(pc, out=pad_off[1:])
    P = int(pad_off[-1])
    assert P <= N_CORES * E_C, (P, N_CORES * E_C)

    order = np.argsort(b, kind="stable")
    seg_start = np.zeros(N_BASINS, np.int64)
    np.cumsum(counts[:-1], out=seg_start[1:])
    bs = b[order]
    dst = pad_off[bs] + (np.arange(n, dtype=np.int64) - seg_start[bs])

    yt_pad = np.zeros(N_CORES * E_C, dtype=BF16_NP)
    yp_pad = np.zeros(N_CORES * E_C, dtype=BF16_NP)
    yt_pad[dst] = y_true[order].astype(BF16_NP)
    yp_pad[dst] = y_pred[order].astype(BF16_NP)
    yt_pad = yt_pad.reshape(N_CORES, E_C)
    yp_pad = yp_pad.reshape(N_CORES, E_C)

    in_maps = [{"yt": yt_pad[c], "yp": yp_pad[c]} for c in range(N_CORES)]

    # basin of every global row (pad rows -> N_BASINS, dropped later)
    row_basin = np.full(N_CORES * R_C, N_BASINS, np.int64)
    rb = np.repeat(np.arange(N_BASINS), pc // W)
    row_basin[: rb.shape[0]] = rb
    return in_maps, (counts, row_basin)


def _finish(results, ctx):
    counts, row_basin = ctx
    rmap = _row_map()
    rows = np.empty((3, N_CORES * R_C), np.float64)
    for c in range(N_CORES):
        arr = np.asarray(results[c]["out"], np.float64).reshape(4, 128, J)
        sl = slice(c * R_C, (c + 1) * R_C)
        for s, plane in enumerate((arr[0] + arr[1], arr[2], arr[3])):
            dest = np.empty(R_C, np.float64)
            dest[rmap.ravel()] = plane.ravel()
            rows[s, sl] = dest
    s_t = np.bincount(row_basin, weights=rows[0], minlength=N_BASINS + 1)[
        :N_BASINS
    ]
    s_t2 = np.bincount(row_basin, weights=rows[1], minlength=N_BASINS + 1)[
        :N_BASINS
    ]
    s_d2 = np.bincount(row_basin, weights=rows[2], minlength=N_BASINS + 1)[
        :N_BASINS
    ]
    cnt = counts.astype(np.float64)
    ss_tot = s_t2 - s_t * s_t / cnt
    nse = 1.0 - s_d2 / (ss_tot + EPS)
    return np.float32(nse.mean())


def kernel(y_pred, y_true, basin):
    in_maps, ctx = _prepare(y_pred, y_true, basin)
    res = run_bass_kernel_spmd(_get_nc(), in_maps, list(range(N_CORES)))
    return _finish(res.results, ctx)
